# revision 27
# baseline (speedup 1.0000x reference)
"""Trainium2 Bass kernel for nn_NeuralMemory (retrieve forward pass).

Computes, for x [B, S, D] (flattened to [T, D]):
    q   = x @ wq + bq
    qn  = LayerNorm(q)               (no scale/bias, eps=1e-5)
    h   = qn
    for i in 0..2:  h = silu(h @ mlp_w[i] + mlp_b[i])
    y   = h @ (mlp_w[3] @ w_out) + (mlp_b[3] @ w_out + b_out)
          (layer 3 is linear, so it is folded into w_out on the host;
           the straight-through term is 0 in the forward pass)

Strategy: data-parallel over the 8 NeuronCores (2048 tokens each), all
matmuls in bf16 (1 cyc/row on the PE, end-to-end max rel err ~7e-3 vs
the 2e-2 gate). 5 matmul passes = 655k PE cycles (~276 us at 2.4 GHz
incl. issue overhead); steady state runs at the 216 ns/MM roofline, so
the optimization surface is the startup/tail edges:

- Phase A (q = x@wq, token-major via the lhsT trick) runs in 4 blocks
  of 4 token-tiles with an output-column split (g in {0,1}), and the
  g0 subloop iterates kc-INNER across the 4 accumulators: each 256KB
  (x, wq) kc-pair that lands unlocks 8 matmuls. The DMA fabric ramps
  slowly over the first ~10us (~150-250 GB/s before settling at ~420),
  so matching consumption order + granularity to the arrival stream is
  what closes the old 14-29us PE-starve window (which also caused a
  10us HAM re-throttle at half clock).
- Queue discipline: the HWDGE ring holds ~4 in-flight DMAs and posts
  beyond that BLOCK the posting engine, so scalar(ACT) posts ONLY the
  1MB of wq g0 pairs (its LN applies must not sit behind blocked
  posts); sync posts x pairs, wq g1, x q1-3, and the 8MB of mlp/out
  weights interleaved between phase-A blocks where they block nothing.
- A short burst of junk matmuls covers the ~5us from preamble end to
  first-data arrival so the PE clock is warm when real work starts.
- LayerNorm stats run on the DVE from PSUM (magic-constant rsqrt +
  Newton), the (q-mu)*rs apply on ACT from PSUM, qn goes feature-major
  via the DMA-transpose XBAR on the sync queue.
- 3 MLP layers contract feature-major with silu+bias on ACT; the
  folded output matmul lands token-major via the lhsT trick.
- PSUM is one 8-bank ring shared by all phases (was 6).
- y is stored [2, tpc, 512] so each output half is a single contiguous
  128KB DMA posted right after its ACT copy; the host re-concatenates.
  This shortens the post-last-matmul tail.

Weight queues: x (+ transposes + y) on the sync HWDGE queue; wq halves
then mw0-2 + wc on the scalar HWDGE queue; tiny mb/bq/bc on gpsimd.
"""
from contextlib import ExitStack

import numpy as np
import ml_dtypes

import concourse.bass as bass
import concourse.mybir as mybir
import concourse.tile as tile
from concourse.bass_utils import run_bass_kernel_spmd

D = 1024
P = 128
KC = D // P          # 8 feature chunks of 128
EPS = 1e-5
N_CORES = 8
F32 = mybir.dt.float32
BF = mybir.dt.bfloat16
AF = mybir.ActivationFunctionType
BF_NP = ml_dtypes.bfloat16

N_JUNK = 18          # PE warm-up matmuls (cover preamble->first-data)

# ---------------------------------------------------------------------------
# Walrus in this container accepts at most 1 semaphore wait per instruction.
# Tile emits more; split the extras onto preceding same-engine NOPs (the
# engine executes in order, so waiting on an earlier NOP is equivalent).
MAX_WAITS = 1


def _legalize_waits(nc, max_waits: int = MAX_WAITS) -> int:
    n_split = 0
    for f in nc.m.functions:
        for bb in f.blocks:
            insts = bb.instructions
            new = []
            for inst in insts:
                si = getattr(inst, "sync_info", None)
                waits = list(si.on_wait) if si is not None and si.on_wait else []
                if len(waits) > max_waits:
                    extra, keep = waits[:-max_waits], waits[-max_waits:]
                    for ci in range(0, len(extra), max_waits):
                        chunk = extra[ci:ci + max_waits]
                        nop = mybir.InstNoOp(
                            name=f"{inst.name}-ws{n_split}-{ci}",
                            engine=inst.engine,
                            sync_info=mybir.SyncInfo(on_wait=chunk, on_update=[]),
                            bass_nofuse=True,
                        )
                        new.append(nop)
                    inst.sync_info = mybir.SyncInfo(
                        on_wait=keep, on_update=list(si.on_update or [])
                    )
                    n_split += 1
                new.append(inst)
            if len(new) != len(insts):
                insts[:] = new
    return n_split


# ---------------------------------------------------------------------------
def build_nc(tpc: int = 2048, zero_bq: bool = True, zero_bc: bool = True,
             legalize: bool = True) -> bass.Bass:
    """Per-core kernel: x_d [NB, P, KC, 512] (bf16, host-packed) ->
    y [2, tpc, 512] bf16."""
    assert tpc % 512 == 0
    NB = tpc // 512      # 512-token blocks (phase A blocks / matmul groups)
    BJ = 4               # 128-token tiles per block

    nc = bass.Bass("TRN2", debug=False)

    # host-packed: x_d[q, p, kc, t] = x[q*512+t, kc*128+p]
    x_d = nc.dram_tensor("xq", [NB, P, KC, 512], BF, kind="ExternalInput").ap()
    # host-packed: wq_d[g, p, kc, m] = wq[kc*128+p, g*512+m]
    wq_d = nc.dram_tensor("wq", [2, P, KC, 512], BF, kind="ExternalInput").ap()
    mw_d = nc.dram_tensor("mw", [3, D, D], BF, kind="ExternalInput").ap()
    wc_d = nc.dram_tensor("wc", [D, D], BF, kind="ExternalInput").ap()
    mb_d = nc.dram_tensor("mb", [3, D], F32, kind="ExternalInput").ap()
    bq_d = nc.dram_tensor("bq", [D], BF, kind="ExternalInput").ap()
    bc_d = nc.dram_tensor("bc", [D], BF, kind="ExternalInput").ap()
    y_d = nc.dram_tensor("y", [2, tpc, 512], BF, kind="ExternalOutput").ap()

    INT32 = mybir.dt.int32
    with tile.TileContext(nc) as tc, ExitStack() as ctx:
        singles = ctx.enter_context(tc.tile_pool(name="singles", bufs=1))
        p_act = ctx.enter_context(tc.tile_pool(name="acts", bufs=3))
        p_w = ctx.enter_context(tc.tile_pool(name="w", bufs=5))
        p_small = ctx.enter_context(tc.tile_pool(name="small", bufs=8))
        p_st = ctx.enter_context(tc.tile_pool(name="stats", bufs=4))
        p_qn = ctx.enter_context(tc.tile_pool(name="qn", bufs=8))
        p_o = ctx.enter_context(tc.tile_pool(name="o", bufs=2))
        psum = ctx.enter_context(tc.tile_pool(name="ps", bufs=8, space="PSUM"))

        def ps_tile():
            return psum.tile([P, 512], F32, name="ps", tag="ps", bufs=8)

        # PE warm-up: junk matmuls on zeroed scratch keep the PE busy from
        # the end of the preamble until the first x/wq chunks land, so the
        # clock is fully ramped when the real chain starts (results unused).
        # Emitted first so the scratch memset is gpsimd's first real op.
        scratch = singles.tile([P, 512], BF, name="scratch")
        nc.gpsimd.memset(scratch[:], 0.0)
        for _ in range(N_JUNK):
            pjunk = ps_tile()
            nc.tensor.matmul(pjunk[:], scratch[:, 0:P], scratch[:],
                             start=True, stop=True)

        # --- input DMAs, earliest-needed first ----------------------------
        # sync queue: x blocks (then qn transposes, then y out).
        # scalar queue: wq halves, then mw0-2, wc.
        # The DMA fabric ramps slowly over the first ~10us (~150-260 GB/s
        # before reaching ~420), so block 0's x and wq g0 are loaded as
        # 256KB kc-pairs: each pair that lands unlocks 8 matmuls of the
        # kc-inner g0 subloop below, keeping the PE fed from ~11us on.
        # Block 0's x + wq g0 (the startup-critical 2MB) spread across FOUR
        # queues (sync/vector/scalar/gpsimd) so enough packets are in
        # flight to hide HBM latency during the slow early fabric ramp.
        x_sb = p_act.tile([P, NB, KC, 512], BF, name="x_sb", tag="act")
        w_q = p_w.tile([P, 2, KC, 512], BF, name="w_sb", tag="w", bufs=5)
        for kp in range(KC // 2):
            nc.sync.dma_start(out=x_sb[:, 0, 2 * kp:2 * kp + 2, :],
                              in_=x_d[0, :, 2 * kp:2 * kp + 2, :])
        # wq g1 rides sync between the block-0 pairs and x q1: once the
        # startup pairs drain, sync gets the full fabric, landing g1
        # right before block 0's g1 subloop needs it.
        nc.sync.dma_start(out=w_q[:, 1, :, :], in_=wq_d[1])
        for q in range(1, NB):
            nc.sync.dma_start(out=x_sb[:, q, :, :], in_=x_d[q])

        # Scalar carries ONLY wq: the HWDGE ring holds ~4 in-flight DMAs
        # and posts beyond that BLOCK the posting engine -- queuing the
        # 8MB of mlp/out weights here would stall ACT (and with it the
        # whole phase-A LN pipeline) behind blocked posts for ~15us.
        # mw0-2 + wc instead post on sync, interleaved between phase-A
        # blocks (emitted in the block loop below), where they are never
        # ahead of anything urgent.
        for kp in range(KC // 2):
            nc.scalar.dma_start(out=w_q[:, 0, 2 * kp:2 * kp + 2, :],
                                in_=wq_d[0, :, 2 * kp:2 * kp + 2, :])
        w_l = []
        for li in range(3):
            w_l.append(p_w.tile([P, KC, D], BF, name="w_sb", tag="w", bufs=5))
        w_c = p_w.tile([P, KC, D], BF, name="w_sb", tag="w", bufs=5)

        def load_late_weights(b):
            if b < 3:
                nc.sync.dma_start(
                    out=w_l[b][:],
                    in_=mw_d[b].rearrange("(kc p) m -> p kc m", p=P),
                )
            else:
                nc.sync.dma_start(
                    out=w_c[:], in_=wc_d.rearrange("(kc p) m -> p kc m", p=P)
                )

        # --- constants / biases -------------------------------------------
        magic_t = singles.tile([P, 1], INT32, name="magic_t")
        nc.gpsimd.memset(magic_t[:], 0x5F3759DF)

        mb_sb = singles.tile([P, 3, KC], F32, name="mb_sb")
        nc.gpsimd.dma_start(out=mb_sb[:], in_=mb_d.rearrange("l (mc p) -> p l mc", p=P))

        ones_col = bq_row = bc_row = None
        if not (zero_bq and zero_bc):
            ones_f32 = singles.tile([1, P], F32, name="ones_f32")
            nc.gpsimd.memset(ones_f32[:], 1.0)
            ones_col = singles.tile([1, P], BF, name="ones_col")
            nc.vector.tensor_copy(ones_col[:], ones_f32[:])
        if not zero_bq:
            bq_row = singles.tile([1, D], BF, name="bq_row")
            nc.gpsimd.dma_start(out=bq_row[:],
                                in_=bq_d.rearrange("(a d) -> a d", a=1))
        if not zero_bc:
            bc_row = singles.tile([1, D], BF, name="bc_row")
            nc.gpsimd.dma_start(out=bc_row[:],
                                in_=bc_d.rearrange("(a d) -> a d", a=1))

        # --- phase A: q = x @ wq (token-major), LN, transpose to f-major ---
        act0 = p_act.tile([P, NB, KC, 512], BF, name="act", tag="act")

        def q_group(b, j, g, pq):
            for kc in range(KC):
                nc.tensor.matmul(
                    pq[:], x_sb[:, b, kc, j * P:(j + 1) * P], w_q[:, g, kc, :],
                    start=(kc == 0), stop=(kc == KC - 1 and zero_bq),
                )
            if not zero_bq:
                nc.tensor.matmul(pq[:], ones_col[:],
                                 bq_row[:, g * 512:(g + 1) * 512],
                                 start=False, stop=True)

        def stage_ln(st, pq0, pq1, b, j):
            nc.vector.bn_stats(out=st[:, 1, :], in_=pq1[:])
            mv = p_small.tile([P, 2], F32, name="mv")
            nc.vector.bn_aggr(out=mv[:], in_=st[:])
            # rsqrt: magic-constant estimate + Newton step (DVE;
            # keeps sqrt off ACT so the silu tables never reload)
            v_t = p_small.tile([P, 1], F32, name="v_t")
            nc.vector.tensor_scalar_add(out=v_t[:], in0=mv[:, 1:2],
                                        scalar1=float(EPS))
            y_t = p_small.tile([P, 1], F32, name="y_t")
            nc.vector.tensor_scalar(
                out=y_t.bitcast(INT32)[:], in0=v_t.bitcast(INT32)[:],
                scalar1=1, scalar2=None,
                op0=mybir.AluOpType.arith_shift_right,
            )
            nc.vector.tensor_sub(y_t.bitcast(INT32)[:], magic_t[:],
                                 y_t.bitcast(INT32)[:])
            c_t = p_small.tile([P, 1], F32, name="c_t")
            for _ in range(1):
                nc.vector.tensor_mul(c_t[:], y_t[:], y_t[:])
                nc.vector.tensor_mul(c_t[:], c_t[:], v_t[:])
                nc.vector.tensor_scalar(
                    out=c_t[:], in0=c_t[:],
                    scalar1=-0.5, scalar2=1.5,
                    op0=mybir.AluOpType.mult, op1=mybir.AluOpType.add,
                )
                nc.vector.tensor_mul(y_t[:], y_t[:], c_t[:])
            nmurs = p_small.tile([P, 1], F32, name="nmurs")
            nc.vector.tensor_scalar(
                out=nmurs[:], in0=mv[:, 0:1], scalar1=y_t[:], scalar2=-1.0,
                op0=mybir.AluOpType.mult, op1=mybir.AluOpType.mult,
            )
            # qn = (q - mu) * rs == rs * q + (-mu*rs), on ACT from PSUM
            qn = p_qn.tile([P, D], BF, name="qn_tm", tag="qn", bufs=8)
            nc.scalar.activation(out=qn[:, 0:512], in_=pq0[:], func=AF.Identity,
                                 bias=nmurs[:], scale=y_t[:])
            nc.scalar.activation(out=qn[:, 512:1024], in_=pq1[:], func=AF.Identity,
                                 bias=nmurs[:], scale=y_t[:])
            # feature-major via the DMA-transpose XBAR on the sync queue
            # (lands in the standard kc*128+p layout); frees the PE of
            # transposes and ACT of the PSUM copybacks entirely.
            nc.sync.dma_start(out=act0[:, b, :, j * P:(j + 1) * P], in_=qn[:],
                              transpose=True)

        # Per block b: g=0 matmuls for the 4 token-tiles (needs only x block
        # b + wq half 0), then per tile the g=1 matmuls + LN chain. The g0
        # subloop runs kc-inner across the 4 pq0 accumulators so each
        # arriving 256KB kc-pair unlocks 8 matmuls during the slow DMA
        # ramp. The four pq0 tiles stay live across g0 (4 PSUM banks); pq1
        # rotates through the other half of the ring.
        for b in range(NB):
            pq0_l = [ps_tile() for _ in range(BJ)]
            for kc in range(KC):
                for j in range(BJ):
                    nc.tensor.matmul(
                        pq0_l[j][:], x_sb[:, b, kc, j * P:(j + 1) * P],
                        w_q[:, 0, kc, :],
                        start=(kc == 0), stop=(kc == KC - 1 and zero_bq),
                    )
            st_l = []
            for j in range(BJ):
                if not zero_bq:
                    nc.tensor.matmul(pq0_l[j][:], ones_col[:],
                                     bq_row[:, 0:512], start=False, stop=True)
                st = p_st.tile([P, 2, 6], F32, name="stats")
                nc.vector.bn_stats(out=st[:, 0, :], in_=pq0_l[j][:])
                st_l.append(st)
            for j in range(BJ):
                pq1 = ps_tile()
                q_group(b, j, 1, pq1)
                stage_ln(st_l[j], pq0_l[j], pq1, b, j)
            load_late_weights(b)

        # --- phase C: 3 silu MLP layers, feature-major --------------------
        cur = act0
        for li in range(3):
            nxt = p_act.tile([P, NB, KC, 512], BF, name="act", tag="act")
            w_sb = w_l[li]
            for q in range(NB):
                for mc in range(KC):
                    pm = ps_tile()
                    for kc in range(KC):
                        nc.tensor.matmul(
                            pm[:], w_sb[:, kc, mc * P:(mc + 1) * P],
                            cur[:, q, kc, :],
                            start=(kc == 0), stop=(kc == KC - 1),
                        )
                    nc.scalar.activation(
                        out=nxt[:, q, mc, :], in_=pm[:],
                        func=AF.Silu, bias=mb_sb[:, li, mc:mc + 1],
                    )
            cur = nxt

        # --- final: y = h @ wc (+ bc), token-major via lhsT trick ----------
        # Each 512-col half is copied (ACT) and DMA'd out on its own so the
        # post-last-matmul tail is one 128KB contiguous transfer.
        last = NB * BJ - 1
        for ts in range(NB * BJ):
            q, j = ts // BJ, ts % BJ
            o_tm = p_o.tile([P, D], BF, name="o_tm", tag="o")
            # The very last tile's second half is computed as two 256-col
            # pieces so the post-last-matmul tail (copy + DMA + receipt)
            # covers only 64KB.
            pieces = ([(0, 512), (512, 1024)] if ts != last
                      else [(0, 512), (512, 768), (768, 1024)])
            for c0, c1 in pieces:
                po = ps_tile()
                for kc in range(KC):
                    nc.tensor.matmul(
                        po[:, 0:c1 - c0], cur[:, q, kc, j * P:(j + 1) * P],
                        w_c[:, kc, c0:c1],
                        start=(kc == 0), stop=(kc == KC - 1 and zero_bc),
                    )
                if not zero_bc:
                    nc.tensor.matmul(po[:, 0:c1 - c0], ones_col[:],
                                     bc_row[:, c0:c1], start=False, stop=True)
                nc.scalar.copy(o_tm[:, c0:c1], po[:, 0:c1 - c0])
                nh = c0 // 512
                nc.sync.dma_start(
                    out=y_d[nh, ts * P:(ts + 1) * P, c0 - 512 * nh:c1 - 512 * nh],
                    in_=o_tm[:, c0:c1])

    if legalize:
        _legalize_waits(nc)
    return nc


# ---------------------------------------------------------------------------
_NC_CACHE: dict = {}
TRACE = False
LAST_RESULT = None


def kernel(x, wq, bq, mlp_w, mlp_b, w_out, b_out):
    x = np.asarray(x, dtype=np.float32)
    orig_shape = x.shape
    xf = np.ascontiguousarray(x.reshape(-1, D))
    T = xf.shape[0]
    assert T % N_CORES == 0
    tpc = T // N_CORES
    NB = tpc // 512

    mlp_w = np.asarray(mlp_w, np.float32)
    mlp_b = np.asarray(mlp_b, np.float32)
    w_out64 = np.asarray(w_out, np.float64)
    wc = (mlp_w[3].astype(np.float64) @ w_out64).astype(np.float32)
    bc = (mlp_b[3].astype(np.float64) @ w_out64
          + np.asarray(b_out, np.float64)).astype(np.float32)
    zero_bq = not np.any(np.asarray(bq))
    zero_bc = not np.any(bc)

    key = (tpc, zero_bq, zero_bc)
    if key not in _NC_CACHE:
        _NC_CACHE[key] = build_nc(tpc, zero_bq, zero_bc)
    nc = _NC_CACHE[key]

    xbf = xf.astype(BF_NP)
    # wq packed as [g, p, kc, m]: wq_d[g, p, kc, m] = wq[kc*128+p, g*512+m]
    wq_bf = np.asarray(wq, np.float32).astype(BF_NP)
    wq_pack = np.ascontiguousarray(
        wq_bf.reshape(KC, P, 2, 512).transpose(2, 1, 0, 3)
    )
    shared = {
        "wq": wq_pack,
        "mw": np.ascontiguousarray(mlp_w[:3]).astype(BF_NP),
        "wc": wc.astype(BF_NP),
        "mb": np.ascontiguousarray(mlp_b[:3]),
        "bq": np.asarray(bq, np.float32).astype(BF_NP),
        "bc": bc.astype(BF_NP),
    }
    # x packed per core as [q, p, kc, t]: x_d[q, p, kc, t] = x[q*512+t, kc*128+p]
    in_maps = []
    for c in range(N_CORES):
        xc = xbf[c * tpc:(c + 1) * tpc]
        xq = np.ascontiguousarray(
            xc.reshape(NB, 512, KC, P).transpose(0, 3, 2, 1)
        )
        in_maps.append({"xq": xq, **shared})
    try:
        res = run_bass_kernel_spmd(nc, in_maps, list(range(N_CORES)), trace=TRACE)
    except Exception:
        # transient device errors (NRT_EXEC_UNIT_UNRECOVERABLE) recover on retry
        res = run_bass_kernel_spmd(nc, in_maps, list(range(N_CORES)), trace=TRACE)
    global LAST_RESULT
    LAST_RESULT = res
    y = np.concatenate(
        [
            np.concatenate(
                [res.results[c]["y"][0], res.results[c]["y"][1]], axis=1
            ).astype(np.float32)
            for c in range(N_CORES)
        ],
        axis=0,
    )
    return y.reshape(orig_shape)
